# revision 18
# baseline (speedup 1.0000x reference)
"""Trainium2 Bass kernel for BidirectionalCrossModalCausalAttention.

Shapes (hardcoded): B=64, S=4, C=1280, HID=256, H=W=32.
Sharding: data-parallel over batch: 8 samples per NeuronCore, weights replicated.

Math (per sample b, x = visual_features[b] as (C, HW)):
  k = Wk @ x + bk            (HID, HW)   } one stacked fp32r GEMM with
  v = Wv @ x + bv            (HID, HW)   } lhsT = [Wk;Wv]^T  (C, 2*HID)
  scores = (q . k) / (max(|q|,eps) * max(|k|,eps))   per hw
  attn = softmax(scores over hw)
  pooled = sum_hw v*attn ;  visual_vector = pooled @ (Wp.T/1024) + bp
  gap-> MLP -> softmax -> sensor_weights ; recal = sensor * sensor_weights
"""
import numpy as np
from contextlib import ExitStack

import concourse.bass as bass
import concourse.tile as tile
from concourse import bacc, mybir
from concourse.bass_utils import run_bass_kernel_spmd

B, S, C, HID, H, W = 64, 4, 1280, 256, 32, 32
HW = H * W
NCORES = 8
BS = B // NCORES          # samples per core
NK = C // 128             # contraction tiles of main GEMM
NM = (2 * HID) // 128     # 4 output row-tiles (2 for k, 2 for v)
EPS = 1e-8
F32 = mybir.dt.float32
F32R = mybir.dt.float32r
AX = mybir.AxisListType.X
ALU = mybir.AluOpType
AF = mybir.ActivationFunctionType

_CACHE = {}


def _build(debug_taps=False):
    nc = bacc.Bacc("TRN2", target_bir_lowering=False, debug=False, num_devices=NCORES)

    # ---- DRAM parameters (per-core shard + replicated weights) ----
    d_x = nc.dram_tensor("x", [BS, C, HW], F32, kind="ExternalInput").ap()
    d_sensor = nc.dram_tensor("sensor", [BS, S], F32, kind="ExternalInput").ap()
    d_sensorT = nc.dram_tensor("sensorT", [S, BS], F32, kind="ExternalInput").ap()
    d_wallT = nc.dram_tensor("wallT", [C, 2 * HID], F32, kind="ExternalInput").ap()
    d_wq = nc.dram_tensor("wq", [S, HID], F32, kind="ExternalInput").ap()
    d_w1s = nc.dram_tensor("w1s", [C, HID], F32, kind="ExternalInput").ap()
    d_w2 = nc.dram_tensor("w2", [HID, S], F32, kind="ExternalInput").ap()
    d_wpTs = nc.dram_tensor("wpTs", [HID, C], F32, kind="ExternalInput").ap()
    d_bk = nc.dram_tensor("bk", [HID, 1], F32, kind="ExternalInput").ap()
    d_bv = nc.dram_tensor("bv", [HID, 1], F32, kind="ExternalInput").ap()
    d_b1 = nc.dram_tensor("b1", [HID, 1], F32, kind="ExternalInput").ap()
    d_bq_col = nc.dram_tensor("bq_col", [HID, 1], F32, kind="ExternalInput").ap()
    d_bq_row = nc.dram_tensor("bq_row", [1, HID], F32, kind="ExternalInput").ap()
    d_bp_row = nc.dram_tensor("bp_row", [1, C], F32, kind="ExternalInput").ap()
    d_b2_row = nc.dram_tensor("b2_row", [1, S], F32, kind="ExternalInput").ap()
    d_ones_col = nc.dram_tensor("ones_col", [1, 128], F32, kind="ExternalInput").ap()
    d_ones_row = nc.dram_tensor("ones_row", [1, BS], F32, kind="ExternalInput").ap()
    d_ones_ck = nc.dram_tensor("ones_ck", [128, 1], F32, kind="ExternalInput").ap()

    d_vtmp = nc.dram_tensor("vtmp", [BS, 2, 128, HW], F32).ap()
    d_attn = nc.dram_tensor("attn", [BS, HW], F32, kind="ExternalOutput").ap()
    d_vv = nc.dram_tensor("vv", [BS, C], F32, kind="ExternalOutput").ap()
    d_sw = nc.dram_tensor("sw", [BS, S], F32, kind="ExternalOutput").ap()
    d_recal = nc.dram_tensor("recal", [BS, S], F32, kind="ExternalOutput").ap()
    d_dbg = {}
    if debug_taps:
        for nm, sh in [("qk_all", [BS, HW]), ("norm2_all", [BS, HW]), ("rk", [BS, HW]),
                       ("scores", [BS, HW]), ("e_t", [BS, HW]), ("rq", [BS, 1]),
                       ("zsum", [BS, 1]), ("q", [BS, HID]), ("qT0", [128, BS]),
                       ("ksb0", [128, HW]), ("k2sb0", [128, HW])]:
            d_dbg[nm] = nc.dram_tensor("dbg_" + nm, sh, F32, kind="ExternalOutput").ap()

    with tile.TileContext(nc) as tc, ExitStack() as ctx:
        P = lambda **kw: ctx.enter_context(tc.tile_pool(**kw))
        wpool = P(name="w", bufs=1)
        xpool = P(name="x", bufs=12)
        kpool = P(name="k", bufs=2)
        vpool = P(name="v", bufs=3)
        spool = P(name="s", bufs=1)     # persistent smalls
        tpool = P(name="t", bufs=3)     # transient smalls
        ps_main = P(name="pm", bufs=2, space="PSUM")
        ps_small = P(name="psm", bufs=2, space="PSUM")
        ps_med = P(name="pmd", bufs=2, space="PSUM")

        mm = nc.tensor.matmul
        act = nc.scalar.activation
        dve = nc.vector

        # ---- weights into SBUF ----
        wall = []
        for k in range(NK):
            t = wpool.tile([128, 2 * HID], F32R, name=f"wall_{k}")
            nc.sync.dma_start(out=t[:], in_=d_wallT[k * 128:(k + 1) * 128, :].bitcast(F32R))
            wall.append(t)
        w1s = []
        for k in range(NK):
            t = wpool.tile([128, HID], F32R, name=f"w1s_{k}")
            nc.sync.dma_start(out=t[:], in_=d_w1s[k * 128:(k + 1) * 128, :].bitcast(F32R))
            w1s.append(t)
        wpTs = []
        for j in range(2):
            t = wpool.tile([128, C], F32R, name=f"wpTs_{j}")
            nc.sync.dma_start(out=t[:], in_=d_wpTs[j * 128:(j + 1) * 128, :].bitcast(F32R))
            wpTs.append(t)
        w2t = []
        for j in range(2):
            t = wpool.tile([128, S], F32R, name=f"w2t_{j}")
            nc.sync.dma_start(out=t[:], in_=d_w2[j * 128:(j + 1) * 128, :].bitcast(F32R))
            w2t.append(t)
        wq_t = wpool.tile([S, HID], F32R, name="wq_t")
        nc.sync.dma_start(out=wq_t[:], in_=d_wq.bitcast(F32R))
        sensorT_t = wpool.tile([S, BS], F32R, name="sensorT_t")
        nc.sync.dma_start(out=sensorT_t[:], in_=d_sensorT.bitcast(F32R))
        sensor_t = wpool.tile([BS, S], F32, name="sensor_t")
        nc.sync.dma_start(out=sensor_t[:], in_=d_sensor)
        bq_row = wpool.tile([1, HID], F32R, name="bq_row")
        nc.sync.dma_start(out=bq_row[:], in_=d_bq_row.bitcast(F32R))
        bp_row = wpool.tile([1, C], F32R, name="bp_row")
        nc.sync.dma_start(out=bp_row[:], in_=d_bp_row.bitcast(F32R))
        b2_row = wpool.tile([1, S], F32R, name="b2_row")
        nc.sync.dma_start(out=b2_row[:], in_=d_b2_row.bitcast(F32R))
        bk_c, bv_c, b1_c, bq_c = [], [], [], []
        for j in range(2):
            for lst, src, nm in ((bk_c, d_bk, "bk"), (bv_c, d_bv, "bv"), (b1_c, d_b1, "b1"), (bq_c, d_bq_col, "bq")):
                t = wpool.tile([128, 1], F32, name=f"{nm}_c{j}")
                nc.sync.dma_start(out=t[:], in_=src[j * 128:(j + 1) * 128, :])
                lst.append(t)
        ones_col = wpool.tile([1, 128], F32R, name="ones_col")
        nc.sync.dma_start(out=ones_col[:], in_=d_ones_col.bitcast(F32R))
        ones_row = wpool.tile([1, BS], F32R, name="ones_row")
        nc.sync.dma_start(out=ones_row[:], in_=d_ones_row.bitcast(F32R))
        ones_ck = wpool.tile([128, 1], F32R, name="ones_ck")
        nc.sync.dma_start(out=ones_ck[:], in_=d_ones_ck.bitcast(F32R))

        # ---- q path (batched over samples) ----
        q_ps = ps_small.tile([BS, HID], F32, tag="small")
        mm(out=q_ps[:], lhsT=sensorT_t[:], rhs=wq_t[:], start=True, stop=False)
        mm(out=q_ps[:], lhsT=ones_row[:], rhs=bq_row[:], start=False, stop=True)
        q2_scratch = tpool.tile([BS, HID], F32, tag="tq")
        q2sum = spool.tile([BS, 1], F32)
        act(q2_scratch[:], q_ps[:], AF.Square, accum_out=q2sum[:])
        if debug_taps:
            nc.sync.dma_start(out=d_dbg["q"], in_=q2_scratch[:])  # holds square(q)
        qnorm = spool.tile([BS, 1], F32)
        act(qnorm[:], q2sum[:], AF.Sqrt)
        qnorm_m = spool.tile([BS, 1], F32)
        dve.tensor_scalar_max(qnorm_m[:], qnorm[:], EPS)
        rq = spool.tile([BS, 1], F32)
        dve.reciprocal(rq[:], qnorm_m[:])
        qT_r = []
        for j in range(2):
            p = ps_med.tile([128, BS], F32, tag="med")
            mm(out=p[:], lhsT=wq_t[:, j * 128:(j + 1) * 128], rhs=sensorT_t[:], start=True, stop=True)
            t = spool.tile([128, BS], F32R, name=f"qT_r{j}")
            act(t[:], p[:], AF.Identity, bias=bq_c[j][:])
            qT_r.append(t)

        # ---- persistent accumulators ----
        qk_all = spool.tile([BS, HW], F32)
        norm2_all = spool.tile([BS, HW], F32)
        gapT = [spool.tile([128, BS], F32, name=f"gapT_{k}") for k in range(NK)]

        # ---- per-sample main GEMM + derived ----
        for s in range(BS):
            xt = [xpool.tile([128, HW], F32R, name=f"xt_{s}_{k}", tag="xt") for k in range(NK)]
            for k in range(NK):
                nc.sync.dma_start(out=xt[k][:], in_=d_x[s, k * 128:(k + 1) * 128, :].bitcast(F32R))
            ks, k2s = [], []
            for m in range(NM):
                acc = ps_main.tile([128, HW], F32, tag="main")
                for k in range(NK):
                    for n in range(2):
                        mm(out=acc[:, n * 512:(n + 1) * 512],
                           lhsT=wall[k][:, m * 128:(m + 1) * 128],
                           rhs=xt[k][:, n * 512:(n + 1) * 512],
                           start=(k == 0), stop=(k == NK - 1))
                if m < 2:
                    t1 = kpool.tile([128, HW], F32R, name=f"ksb_{s}_{m}", tag="ksb")
                    act(t1[:], acc[:], AF.Identity, bias=bk_c[m][:])
                    t2 = kpool.tile([128, HW], F32R, name=f"k2sb_{s}_{m}", tag="k2sb")
                    act(t2[:], acc[:], AF.Square, bias=bk_c[m][:])
                    ks.append(t1)
                    k2s.append(t2)
                    if debug_taps and s == 0 and m == 0:
                        nc.sync.dma_start(out=d_dbg["ksb0"], in_=t1[:].bitcast(F32))
                        nc.sync.dma_start(out=d_dbg["k2sb0"], in_=t2[:].bitcast(F32))
                else:
                    j = m - 2
                    t = vpool.tile([128, HW], F32, name=f"vsb_{s}_{j}", tag="vsb")
                    act(t[:], acc[:], AF.Identity, bias=bv_c[j][:])
                    nc.sync.dma_start(out=d_vtmp[s, j], in_=t[:])
            for qi, (dst, rhs_t) in enumerate(((qk_all, ks), (norm2_all, k2s))):
                row = tpool.tile([1, HW], F32, tag="row", name=f"row_{s}_{qi}")
                for n in range(2):
                    p = ps_small.tile([1, 512], F32, tag="small", name=f"rps_{s}_{n}_{qi}")
                    for j in range(2):
                        lhs = qT_r[j][:, s:s + 1] if qi == 0 else ones_ck[:]
                        mm(out=p[:], lhsT=lhs,
                           rhs=rhs_t[j][:, n * 512:(n + 1) * 512],
                           start=(j == 0), stop=(j == 1))
                    act(row[0:1, n * 512:(n + 1) * 512], p[:], AF.Copy)
                nc.sync.dma_start(out=dst[s:s + 1, :], in_=row[:])
            for k in range(NK):
                dve.tensor_reduce(gapT[k][:, s:s + 1], xt[k][:].bitcast(F32), AX, ALU.add)

        # ---- scores -> attn (batched (BS, HW)) ----
        knorm = tpool.tile([BS, HW], F32, tag="tb")
        act(knorm[:], norm2_all[:], AF.Sqrt)
        knorm_m = tpool.tile([BS, HW], F32, tag="tb")
        dve.tensor_scalar_max(knorm_m[:], knorm[:], EPS)
        rk = tpool.tile([BS, HW], F32, tag="tb")
        dve.reciprocal(rk[:], knorm_m[:])
        scores = tpool.tile([BS, HW], F32, tag="tb")
        dve.tensor_tensor(scores[:], qk_all[:], rk[:], ALU.mult)
        maxs = spool.tile([BS, 1], F32)
        dve.tensor_reduce(maxs[:], scores[:], AX, ALU.max)
        neg_rq = spool.tile([BS, 1], F32)
        dve.tensor_scalar_mul(neg_rq[:], rq[:], -1.0)
        bias2 = spool.tile([BS, 1], F32)
        dve.tensor_tensor(bias2[:], maxs[:], neg_rq[:], ALU.mult)
        e_t = tpool.tile([BS, HW], F32, tag="tb")
        zsum = spool.tile([BS, 1], F32)
        act(e_t[:], scores[:], AF.Exp, bias=bias2[:], scale=rq[:], accum_out=zsum[:])
        rz = spool.tile([BS, 1], F32)
        dve.reciprocal(rz[:], zsum[:])
        if debug_taps:
            nc.sync.dma_start(out=d_dbg["qk_all"], in_=qk_all[:])
            nc.sync.dma_start(out=d_dbg["norm2_all"], in_=norm2_all[:])
            nc.sync.dma_start(out=d_dbg["rk"], in_=rk[:])
            nc.sync.dma_start(out=d_dbg["scores"], in_=scores[:])
            nc.sync.dma_start(out=d_dbg["e_t"], in_=e_t[:])
            nc.sync.dma_start(out=d_dbg["rq"], in_=rq[:])
            nc.sync.dma_start(out=d_dbg["zsum"], in_=zsum[:])
            nc.sync.dma_start(out=d_dbg["qT0"], in_=qT_r[0][:].bitcast(F32))
        attn_f = tpool.tile([BS, HW], F32, tag="tb")
        act(attn_f[:], e_t[:], AF.Copy, scale=rz[:])
        nc.sync.dma_start(out=d_attn[:, :], in_=attn_f[:])
        # ---- pooled (attn-weighted v reduce) ----
        pooledT = [spool.tile([128, BS], F32, name=f"pooledT_{j}") for j in range(2)]
        for s in range(BS):
            arow = tpool.tile([1, HW], F32R, tag="arow", name=f"attn_row_{s}", bufs=3)
            nc.sync.dma_start(out=arow[:], in_=attn_f[s:s + 1, :].bitcast(F32R))
            bc = ps_main.tile([128, HW], F32, tag="main", name=f"bc_{s}")
            for n in range(2):
                mm(out=bc[:, n * 512:(n + 1) * 512], lhsT=ones_col[:],
                   rhs=arow[0:1, n * 512:(n + 1) * 512], start=True, stop=True)
            for j in range(2):
                vt = vpool.tile([128, HW], F32, name=f"vld_{s}_{j}", tag="vsb")
                nc.sync.dma_start(out=vt[:], in_=d_vtmp[s, j])
                scr = tpool.tile([128, HW], F32, tag="scr", name=f"scr_{s}_{j}", bufs=2)
                dve.tensor_tensor(scr[:], vt[:], bc[:], ALU.mult)
                dve.tensor_reduce(pooledT[j][:, s:s + 1], scr[:], AX, ALU.add)
        pooledT_r = []
        for j in range(2):
            t = spool.tile([128, BS], F32R, name=f"pooledT_r{j}")
            act(t[:], pooledT[j][:], AF.Copy)
            pooledT_r.append(t)

        # ---- visual_vector = pooled @ (Wp.T/1024) + bp ----
        vv_sb = spool.tile([BS, C], F32)
        for nchunk, n0 in ((512, 0), (512, 512), (256, 1024)):
            p = ps_small.tile([BS, 512], F32, tag="small", name=f"vv_ps_{n0}")
            for j in range(2):
                mm(out=p[:, :nchunk], lhsT=pooledT_r[j][:], rhs=wpTs[j][:, n0:n0 + nchunk],
                   start=(j == 0), stop=False)
            mm(out=p[:, :nchunk], lhsT=ones_row[:], rhs=bp_row[:, n0:n0 + nchunk],
               start=False, stop=True)
            act(vv_sb[:, n0:n0 + nchunk], p[:, :nchunk], AF.Copy)
        nc.sync.dma_start(out=d_vv[:, :], in_=vv_sb[:])

        # ---- gap MLP -> sensor weights ----
        gapT_r = []
        for k in range(NK):
            t = spool.tile([128, BS], F32R, name=f"gapT_r{k}")
            act(t[:], gapT[k][:], AF.Copy)
            gapT_r.append(t)
        hiddenT_r = []
        for j in range(2):
            p = ps_med.tile([128, BS], F32, tag="med", name=f"hid_ps{j}")
            for k in range(NK):
                mm(out=p[:], lhsT=w1s[k][:, j * 128:(j + 1) * 128], rhs=gapT_r[k][:],
                   start=(k == 0), stop=(k == NK - 1))
            t = spool.tile([128, BS], F32R, name=f"hiddenT_r{j}")
            act(t[:], p[:], AF.Relu, bias=b1_c[j][:])
            hiddenT_r.append(t)
        lg_ps = ps_small.tile([BS, S], F32, tag="small")
        for j in range(2):
            mm(out=lg_ps[:], lhsT=hiddenT_r[j][:], rhs=w2t[j][:], start=(j == 0), stop=False)
        mm(out=lg_ps[:], lhsT=ones_row[:], rhs=b2_row[:], start=False, stop=True)
        lmax = spool.tile([BS, 1], F32)
        dve.tensor_reduce(lmax[:], lg_ps[:], AX, ALU.max)
        nlmax = spool.tile([BS, 1], F32)
        dve.tensor_scalar_mul(nlmax[:], lmax[:], -1.0)
        le_t = spool.tile([BS, S], F32)
        lz = spool.tile([BS, 1], F32)
        act(le_t[:], lg_ps[:], AF.Exp, bias=nlmax[:], accum_out=lz[:])
        rlz = spool.tile([BS, 1], F32)
        dve.reciprocal(rlz[:], lz[:])
        sw_sb = spool.tile([BS, S], F32)
        act(sw_sb[:], le_t[:], AF.Copy, scale=rlz[:])
        nc.sync.dma_start(out=d_sw[:, :], in_=sw_sb[:])
        recal_sb = spool.tile([BS, S], F32)
        dve.tensor_tensor(recal_sb[:], sensor_t[:], sw_sb[:], ALU.mult)
        nc.sync.dma_start(out=d_recal[:, :], in_=recal_sb[:])

    nc.compile()
    return nc


def _prep_inputs(inputs):
    f = lambda a: np.ascontiguousarray(np.asarray(a, dtype=np.float32))
    sensor = f(inputs["sensor_features"])
    x = f(inputs["visual_features"]).reshape(B, C, HW)
    Wk, Wv = f(inputs["Wk"]), f(inputs["Wv"])
    wallT = np.ascontiguousarray(np.concatenate([Wk.T, Wv.T], axis=1))  # (C, 2*HID)
    shared = {
        "wallT": wallT,
        "wq": f(inputs["Wq"]),
        "w1s": np.ascontiguousarray(f(inputs["W1"]) / HW),
        "w2": f(inputs["W2"]),
        "wpTs": np.ascontiguousarray(f(inputs["Wp"]).T / HW),
        "bk": f(inputs["bk"]).reshape(HID, 1),
        "bv": f(inputs["bv"]).reshape(HID, 1),
        "b1": f(inputs["b1"]).reshape(HID, 1),
        "bq_col": f(inputs["bq"]).reshape(HID, 1),
        "bq_row": f(inputs["bq"]).reshape(1, HID),
        "bp_row": f(inputs["bp"]).reshape(1, C),
        "b2_row": f(inputs["b2"]).reshape(1, S),
        "ones_col": np.ones((1, 128), np.float32),
        "ones_row": np.ones((1, BS), np.float32),
        "ones_ck": np.ones((128, 1), np.float32),
    }
    in_maps = []
    for i in range(NCORES):
        sl = slice(i * BS, (i + 1) * BS)
        m = dict(shared)
        m["x"] = np.ascontiguousarray(x[sl])
        m["sensor"] = np.ascontiguousarray(sensor[sl])
        m["sensorT"] = np.ascontiguousarray(sensor[sl].T)
        in_maps.append(m)
    return in_maps


def kernel(**inputs):
    if "nc" not in _CACHE:
        _CACHE["nc"] = _build()
    nc = _CACHE["nc"]
    in_maps = _prep_inputs(inputs)
    res = run_bass_kernel_spmd(nc, in_maps, list(range(NCORES))).results
    vv = np.concatenate([r["vv"] for r in res], axis=0)
    recal = np.concatenate([r["recal"] for r in res], axis=0)
    attn = np.concatenate([r["attn"] for r in res], axis=0).reshape(B, 1, H, W)
    sw = np.concatenate([r["sw"] for r in res], axis=0)
    return (vv, recal, attn, sw)


# revision 20
# speedup vs baseline: 1.1360x; 1.1360x over previous
"""Trainium2 Bass kernel for BidirectionalCrossModalCausalAttention.

Shapes (hardcoded): B=64, S=4, C=1280, HID=256, H=W=32.
Sharding: data-parallel over batch: 8 samples per NeuronCore, weights replicated.

Math (per sample b, x = visual_features[b] as (C, HW)):
  k = Wk @ x + bk            (HID, HW)   } one stacked fp32r GEMM with
  v = Wv @ x + bv            (HID, HW)   } lhsT = [Wk;Wv]^T  (C, 2*HID)
  scores = (q . k) / (max(|q|,eps) * max(|k|,eps))   per hw
  attn = softmax(scores over hw)
  pooled = sum_hw v*attn ;  visual_vector = pooled @ (Wp.T/1024) + bp
  gap-> MLP -> softmax -> sensor_weights ; recal = sensor * sensor_weights
"""
import numpy as np
from contextlib import ExitStack

import concourse.bass as bass
import concourse.tile as tile
from concourse import bacc, mybir
from concourse.bass_utils import run_bass_kernel_spmd

B, S, C, HID, H, W = 64, 4, 1280, 256, 32, 32
HW = H * W
NCORES = 8
BS = B // NCORES          # samples per core
NK = C // 128             # contraction tiles of main GEMM
NM = (2 * HID) // 128     # 4 output row-tiles (2 for k, 2 for v)
EPS = 1e-8
F32 = mybir.dt.float32
F32R = mybir.dt.float32r
AX = mybir.AxisListType.X
ALU = mybir.AluOpType
AF = mybir.ActivationFunctionType

_CACHE = {}


def _build(debug_taps=False):
    nc = bacc.Bacc("TRN2", target_bir_lowering=False, debug=False, num_devices=NCORES)

    # ---- DRAM parameters (per-core shard + replicated weights) ----
    d_x = nc.dram_tensor("x", [BS, C, HW], F32, kind="ExternalInput").ap()
    d_sensor = nc.dram_tensor("sensor", [BS, S], F32, kind="ExternalInput").ap()
    d_sensorT = nc.dram_tensor("sensorT", [S, BS], F32, kind="ExternalInput").ap()
    d_wallT = nc.dram_tensor("wallT", [C, 2 * HID], F32, kind="ExternalInput").ap()
    d_wq = nc.dram_tensor("wq", [S, HID], F32, kind="ExternalInput").ap()
    d_w1s = nc.dram_tensor("w1s", [C, HID], F32, kind="ExternalInput").ap()
    d_w2 = nc.dram_tensor("w2", [HID, S], F32, kind="ExternalInput").ap()
    d_wpTs = nc.dram_tensor("wpTs", [HID, C], F32, kind="ExternalInput").ap()
    d_bk = nc.dram_tensor("bk", [HID, 1], F32, kind="ExternalInput").ap()
    d_bv = nc.dram_tensor("bv", [HID, 1], F32, kind="ExternalInput").ap()
    d_b1 = nc.dram_tensor("b1", [HID, 1], F32, kind="ExternalInput").ap()
    d_bq_col = nc.dram_tensor("bq_col", [HID, 1], F32, kind="ExternalInput").ap()
    d_bq_row = nc.dram_tensor("bq_row", [1, HID], F32, kind="ExternalInput").ap()
    d_bp_row = nc.dram_tensor("bp_row", [1, C], F32, kind="ExternalInput").ap()
    d_b2_row = nc.dram_tensor("b2_row", [1, S], F32, kind="ExternalInput").ap()
    d_ones_col = nc.dram_tensor("ones_col", [1, 128], F32, kind="ExternalInput").ap()
    d_ones_row = nc.dram_tensor("ones_row", [1, BS], F32, kind="ExternalInput").ap()
    d_ones_ck = nc.dram_tensor("ones_ck", [128, 1], F32, kind="ExternalInput").ap()

    d_vtmp = nc.dram_tensor("vtmp", [BS, 2, 128, HW], F32).ap()
    d_attn = nc.dram_tensor("attn", [BS, HW], F32, kind="ExternalOutput").ap()
    d_vv = nc.dram_tensor("vv", [BS, C], F32, kind="ExternalOutput").ap()
    d_sw = nc.dram_tensor("sw", [BS, S], F32, kind="ExternalOutput").ap()
    d_recal = nc.dram_tensor("recal", [BS, S], F32, kind="ExternalOutput").ap()
    d_dbg = {}
    if debug_taps:
        for nm, sh in [("qk_all", [BS, HW]), ("norm2_all", [BS, HW]), ("rk", [BS, HW]),
                       ("scores", [BS, HW]), ("e_t", [BS, HW]), ("rq", [BS, 1]),
                       ("zsum", [BS, 1]), ("q", [BS, HID]), ("qT0", [128, BS]),
                       ("ksb0", [128, HW]), ("k2sb0", [128, HW])]:
            d_dbg[nm] = nc.dram_tensor("dbg_" + nm, sh, F32, kind="ExternalOutput").ap()

    with tile.TileContext(nc) as tc, ExitStack() as ctx:
        P = lambda **kw: ctx.enter_context(tc.tile_pool(**kw))
        wpool = P(name="w", bufs=1)
        xpool = P(name="x", bufs=15)
        kpool = P(name="k", bufs=2)
        vpool = P(name="v", bufs=5)
        spool = P(name="s", bufs=1)     # persistent smalls
        tpool = P(name="t", bufs=3)     # transient smalls
        ps_main = P(name="pm", bufs=2, space="PSUM")
        ps_small = P(name="psm", bufs=4, space="PSUM")
        ps_med = ps_small

        mm = nc.tensor.matmul
        act = nc.scalar.activation
        dve = nc.vector

        # ---- weights into SBUF ----
        wall = []
        for k in range(NK):
            t = wpool.tile([128, 2 * HID], F32R, name=f"wall_{k}")
            nc.sync.dma_start(out=t[:], in_=d_wallT[k * 128:(k + 1) * 128, :].bitcast(F32R))
            wall.append(t)
        wq_t = wpool.tile([S, HID], F32R, name="wq_t")
        nc.sync.dma_start(out=wq_t[:], in_=d_wq.bitcast(F32R))
        sensorT_t = wpool.tile([S, BS], F32R, name="sensorT_t")
        nc.sync.dma_start(out=sensorT_t[:], in_=d_sensorT.bitcast(F32R))
        sensor_t = wpool.tile([BS, S], F32, name="sensor_t")
        nc.sync.dma_start(out=sensor_t[:], in_=d_sensor)
        bq_row = wpool.tile([1, HID], F32R, name="bq_row")
        nc.sync.dma_start(out=bq_row[:], in_=d_bq_row.bitcast(F32R))
        bk_c, bv_c, b1_c, bq_c = [], [], [], []
        for j in range(2):
            for lst, src, nm in ((bk_c, d_bk, "bk"), (bv_c, d_bv, "bv"), (b1_c, d_b1, "b1"), (bq_c, d_bq_col, "bq")):
                t = wpool.tile([128, 1], F32, name=f"{nm}_c{j}")
                nc.sync.dma_start(out=t[:], in_=src[j * 128:(j + 1) * 128, :])
                lst.append(t)
        ones_col = wpool.tile([1, 128], F32R, name="ones_col")
        nc.sync.dma_start(out=ones_col[:], in_=d_ones_col.bitcast(F32R))
        ones_row = wpool.tile([1, BS], F32R, name="ones_row")
        nc.sync.dma_start(out=ones_row[:], in_=d_ones_row.bitcast(F32R))
        ones_ck = wpool.tile([128, 1], F32R, name="ones_ck")
        nc.sync.dma_start(out=ones_ck[:], in_=d_ones_ck.bitcast(F32R))

        # ---- q path (batched over samples) ----
        q_ps = ps_small.tile([BS, HID], F32, tag="small")
        mm(out=q_ps[:], lhsT=sensorT_t[:], rhs=wq_t[:], start=True, stop=False)
        mm(out=q_ps[:], lhsT=ones_row[:], rhs=bq_row[:], start=False, stop=True)
        q2_scratch = tpool.tile([BS, HID], F32, tag="tq")
        q2sum = spool.tile([BS, 1], F32)
        act(q2_scratch[:], q_ps[:], AF.Square, accum_out=q2sum[:])
        if debug_taps:
            nc.sync.dma_start(out=d_dbg["q"], in_=q2_scratch[:])  # holds square(q)
        qnorm = spool.tile([BS, 1], F32)
        act(qnorm[:], q2sum[:], AF.Sqrt)
        qnorm_m = spool.tile([BS, 1], F32)
        dve.tensor_scalar_max(qnorm_m[:], qnorm[:], EPS)
        rq = spool.tile([BS, 1], F32)
        dve.reciprocal(rq[:], qnorm_m[:])
        qT_r = []
        for j in range(2):
            p = ps_med.tile([128, BS], F32, tag="small")
            mm(out=p[:], lhsT=wq_t[:, j * 128:(j + 1) * 128], rhs=sensorT_t[:], start=True, stop=True)
            t = spool.tile([128, BS], F32R, name=f"qT_r{j}")
            act(t[:], p[:], AF.Identity, bias=bq_c[j][:])
            qT_r.append(t)

        # ---- persistent accumulators ----
        qk_all = spool.tile([BS, HW], F32)
        norm2_all = spool.tile([BS, HW], F32)
        gapT = [spool.tile([128, BS], F32, name=f"gapT_{k}") for k in range(NK)]

        # ---- per-sample main GEMM + derived ----
        for s in range(BS):
            xt = [xpool.tile([128, HW], F32R, name=f"xt_{s}_{k}", tag="xt") for k in range(NK)]
            for k in range(NK):
                nc.gpsimd.dma_start(out=xt[k][:], in_=d_x[s, k * 128:(k + 1) * 128, :].bitcast(F32R))
            ks, k2s = [], []
            for m in range(NM):
                acc = ps_main.tile([128, HW], F32, tag="main")
                for k in range(NK):
                    for n in range(2):
                        mm(out=acc[:, n * 512:(n + 1) * 512],
                           lhsT=wall[k][:, m * 128:(m + 1) * 128],
                           rhs=xt[k][:, n * 512:(n + 1) * 512],
                           start=(k == 0), stop=(k == NK - 1))
                if m < 2:
                    t1 = kpool.tile([128, HW], F32R, name=f"ksb_{s}_{m}", tag="ksb")
                    act(t1[:], acc[:], AF.Identity, bias=bk_c[m][:])
                    t2 = kpool.tile([128, HW], F32R, name=f"k2sb_{s}_{m}", tag="k2sb")
                    act(t2[:], acc[:], AF.Square, bias=bk_c[m][:])
                    ks.append(t1)
                    k2s.append(t2)
                    if debug_taps and s == 0 and m == 0:
                        nc.sync.dma_start(out=d_dbg["ksb0"], in_=t1[:].bitcast(F32))
                        nc.sync.dma_start(out=d_dbg["k2sb0"], in_=t2[:].bitcast(F32))
                else:
                    j = m - 2
                    t = vpool.tile([128, HW], F32, name=f"vsb_{s}_{j}", tag="vsb")
                    act(t[:], acc[:], AF.Identity, bias=bv_c[j][:])
                    nc.gpsimd.dma_start(out=d_vtmp[s, j], in_=t[:])
            for qi, (dst, rhs_t) in enumerate(((qk_all, ks), (norm2_all, k2s))):
                row = tpool.tile([1, HW], F32, tag="row", name=f"row_{s}_{qi}", bufs=2)
                for n in range(2):
                    p = ps_small.tile([1, 512], F32, tag="small", name=f"rps_{s}_{n}_{qi}")
                    for j in range(2):
                        lhs = qT_r[j][:, s:s + 1] if qi == 0 else ones_ck[:]
                        mm(out=p[:], lhsT=lhs,
                           rhs=rhs_t[j][:, n * 512:(n + 1) * 512],
                           start=(j == 0), stop=(j == 1))
                    act(row[0:1, n * 512:(n + 1) * 512], p[:], AF.Copy)
                nc.sync.dma_start(out=dst[s:s + 1, :], in_=row[:])
            for k in range(NK):
                dve.tensor_reduce(gapT[k][:, s:s + 1], xt[k][:].bitcast(F32), AX, ALU.add)

        # ---- late weight loads (not needed until epilogue) ----
        w1s, wpTs, w2t = [], [], []
        for k in range(NK):
            t = wpool.tile([128, HID], F32R, name=f"w1s_{k}")
            nc.sync.dma_start(out=t[:], in_=d_w1s[k * 128:(k + 1) * 128, :].bitcast(F32R))
            w1s.append(t)
        for j in range(2):
            t = wpool.tile([128, C], F32R, name=f"wpTs_{j}")
            nc.sync.dma_start(out=t[:], in_=d_wpTs[j * 128:(j + 1) * 128, :].bitcast(F32R))
            wpTs.append(t)
        for j in range(2):
            t = wpool.tile([128, S], F32R, name=f"w2t_{j}")
            nc.sync.dma_start(out=t[:], in_=d_w2[j * 128:(j + 1) * 128, :].bitcast(F32R))
            w2t.append(t)
        bp_row = wpool.tile([1, C], F32R, name="bp_row")
        nc.sync.dma_start(out=bp_row[:], in_=d_bp_row.bitcast(F32R))
        b2_row = wpool.tile([1, S], F32R, name="b2_row")
        nc.sync.dma_start(out=b2_row[:], in_=d_b2_row.bitcast(F32R))

        # ---- scores -> attn (batched (BS, HW)) ----
        knorm = tpool.tile([BS, HW], F32, tag="tb")
        act(knorm[:], norm2_all[:], AF.Sqrt)
        knorm_m = tpool.tile([BS, HW], F32, tag="tb")
        dve.tensor_scalar_max(knorm_m[:], knorm[:], EPS)
        rk = tpool.tile([BS, HW], F32, tag="tb")
        dve.reciprocal(rk[:], knorm_m[:])
        scores = tpool.tile([BS, HW], F32, tag="tb")
        dve.tensor_tensor(scores[:], qk_all[:], rk[:], ALU.mult)
        maxs = spool.tile([BS, 1], F32)
        dve.tensor_reduce(maxs[:], scores[:], AX, ALU.max)
        neg_rq = spool.tile([BS, 1], F32)
        dve.tensor_scalar_mul(neg_rq[:], rq[:], -1.0)
        bias2 = spool.tile([BS, 1], F32)
        dve.tensor_tensor(bias2[:], maxs[:], neg_rq[:], ALU.mult)
        e_t = tpool.tile([BS, HW], F32, tag="tb")
        zsum = spool.tile([BS, 1], F32)
        act(e_t[:], scores[:], AF.Exp, bias=bias2[:], scale=rq[:], accum_out=zsum[:])
        rz = spool.tile([BS, 1], F32)
        dve.reciprocal(rz[:], zsum[:])
        if debug_taps:
            nc.sync.dma_start(out=d_dbg["qk_all"], in_=qk_all[:])
            nc.sync.dma_start(out=d_dbg["norm2_all"], in_=norm2_all[:])
            nc.sync.dma_start(out=d_dbg["rk"], in_=rk[:])
            nc.sync.dma_start(out=d_dbg["scores"], in_=scores[:])
            nc.sync.dma_start(out=d_dbg["e_t"], in_=e_t[:])
            nc.sync.dma_start(out=d_dbg["rq"], in_=rq[:])
            nc.sync.dma_start(out=d_dbg["zsum"], in_=zsum[:])
            nc.sync.dma_start(out=d_dbg["qT0"], in_=qT_r[0][:].bitcast(F32))
        attn_f = tpool.tile([BS, HW], F32, tag="tb")
        act(attn_f[:], e_t[:], AF.Copy, scale=rz[:])
        nc.sync.dma_start(out=d_attn[:, :], in_=attn_f[:])
        # ---- pooled (attn-weighted v reduce) ----
        pooledT = [spool.tile([128, BS], F32, name=f"pooledT_{j}") for j in range(2)]
        for s in range(BS):
            arow = tpool.tile([1, HW], F32R, tag="arow", name=f"attn_row_{s}", bufs=4)
            nc.sync.dma_start(out=arow[:], in_=attn_f[s:s + 1, :].bitcast(F32R))
            bc = ps_main.tile([128, HW], F32, tag="main", name=f"bc_{s}")
            for n in range(2):
                mm(out=bc[:, n * 512:(n + 1) * 512], lhsT=ones_col[:],
                   rhs=arow[0:1, n * 512:(n + 1) * 512], start=True, stop=True)
            for j in range(2):
                vt = vpool.tile([128, HW], F32, name=f"vld_{s}_{j}", tag="vsb")
                nc.gpsimd.dma_start(out=vt[:], in_=d_vtmp[s, j])
                scr = tpool.tile([128, HW], F32, tag="scr", name=f"scr_{s}_{j}", bufs=2)
                dve.tensor_tensor(scr[:], vt[:], bc[:], ALU.mult)
                dve.tensor_reduce(pooledT[j][:, s:s + 1], scr[:], AX, ALU.add)
        pooledT_r = []
        for j in range(2):
            t = spool.tile([128, BS], F32R, name=f"pooledT_r{j}")
            act(t[:], pooledT[j][:], AF.Copy)
            pooledT_r.append(t)

        # ---- visual_vector = pooled @ (Wp.T/1024) + bp ----
        vv_sb = spool.tile([BS, C], F32)
        for nchunk, n0 in ((512, 0), (512, 512), (256, 1024)):
            p = ps_small.tile([BS, 512], F32, tag="small", name=f"vv_ps_{n0}")
            for j in range(2):
                mm(out=p[:, :nchunk], lhsT=pooledT_r[j][:], rhs=wpTs[j][:, n0:n0 + nchunk],
                   start=(j == 0), stop=False)
            mm(out=p[:, :nchunk], lhsT=ones_row[:], rhs=bp_row[:, n0:n0 + nchunk],
               start=False, stop=True)
            act(vv_sb[:, n0:n0 + nchunk], p[:, :nchunk], AF.Copy)
        nc.sync.dma_start(out=d_vv[:, :], in_=vv_sb[:])

        # ---- gap MLP -> sensor weights ----
        gapT_r = []
        for k in range(NK):
            t = spool.tile([128, BS], F32R, name=f"gapT_r{k}")
            act(t[:], gapT[k][:], AF.Copy)
            gapT_r.append(t)
        hiddenT_r = []
        for j in range(2):
            p = ps_med.tile([128, BS], F32, tag="small", name=f"hid_ps{j}")
            for k in range(NK):
                mm(out=p[:], lhsT=w1s[k][:, j * 128:(j + 1) * 128], rhs=gapT_r[k][:],
                   start=(k == 0), stop=(k == NK - 1))
            t = spool.tile([128, BS], F32R, name=f"hiddenT_r{j}")
            act(t[:], p[:], AF.Relu, bias=b1_c[j][:])
            hiddenT_r.append(t)
        lg_ps = ps_small.tile([BS, S], F32, tag="small")
        for j in range(2):
            mm(out=lg_ps[:], lhsT=hiddenT_r[j][:], rhs=w2t[j][:], start=(j == 0), stop=False)
        mm(out=lg_ps[:], lhsT=ones_row[:], rhs=b2_row[:], start=False, stop=True)
        lmax = spool.tile([BS, 1], F32)
        dve.tensor_reduce(lmax[:], lg_ps[:], AX, ALU.max)
        nlmax = spool.tile([BS, 1], F32)
        dve.tensor_scalar_mul(nlmax[:], lmax[:], -1.0)
        le_t = spool.tile([BS, S], F32)
        lz = spool.tile([BS, 1], F32)
        act(le_t[:], lg_ps[:], AF.Exp, bias=nlmax[:], accum_out=lz[:])
        rlz = spool.tile([BS, 1], F32)
        dve.reciprocal(rlz[:], lz[:])
        sw_sb = spool.tile([BS, S], F32)
        act(sw_sb[:], le_t[:], AF.Copy, scale=rlz[:])
        nc.sync.dma_start(out=d_sw[:, :], in_=sw_sb[:])
        recal_sb = spool.tile([BS, S], F32)
        dve.tensor_tensor(recal_sb[:], sensor_t[:], sw_sb[:], ALU.mult)
        nc.sync.dma_start(out=d_recal[:, :], in_=recal_sb[:])

    nc.compile()
    return nc


def _prep_inputs(inputs):
    f = lambda a: np.ascontiguousarray(np.asarray(a, dtype=np.float32))
    sensor = f(inputs["sensor_features"])
    x = f(inputs["visual_features"]).reshape(B, C, HW)
    Wk, Wv = f(inputs["Wk"]), f(inputs["Wv"])
    wallT = np.ascontiguousarray(np.concatenate([Wk.T, Wv.T], axis=1))  # (C, 2*HID)
    shared = {
        "wallT": wallT,
        "wq": f(inputs["Wq"]),
        "w1s": np.ascontiguousarray(f(inputs["W1"]) / HW),
        "w2": f(inputs["W2"]),
        "wpTs": np.ascontiguousarray(f(inputs["Wp"]).T / HW),
        "bk": f(inputs["bk"]).reshape(HID, 1),
        "bv": f(inputs["bv"]).reshape(HID, 1),
        "b1": f(inputs["b1"]).reshape(HID, 1),
        "bq_col": f(inputs["bq"]).reshape(HID, 1),
        "bq_row": f(inputs["bq"]).reshape(1, HID),
        "bp_row": f(inputs["bp"]).reshape(1, C),
        "b2_row": f(inputs["b2"]).reshape(1, S),
        "ones_col": np.ones((1, 128), np.float32),
        "ones_row": np.ones((1, BS), np.float32),
        "ones_ck": np.ones((128, 1), np.float32),
    }
    in_maps = []
    for i in range(NCORES):
        sl = slice(i * BS, (i + 1) * BS)
        m = dict(shared)
        m["x"] = np.ascontiguousarray(x[sl])
        m["sensor"] = np.ascontiguousarray(sensor[sl])
        m["sensorT"] = np.ascontiguousarray(sensor[sl].T)
        in_maps.append(m)
    return in_maps


def kernel(**inputs):
    if "nc" not in _CACHE:
        _CACHE["nc"] = _build()
    nc = _CACHE["nc"]
    in_maps = _prep_inputs(inputs)
    res = run_bass_kernel_spmd(nc, in_maps, list(range(NCORES))).results
    vv = np.concatenate([r["vv"] for r in res], axis=0)
    recal = np.concatenate([r["recal"] for r in res], axis=0)
    attn = np.concatenate([r["attn"] for r in res], axis=0).reshape(B, 1, H, W)
    sw = np.concatenate([r["sw"] for r in res], axis=0)
    return (vv, recal, attn, sw)
